# revision 18
# baseline (speedup 1.0000x reference)
"""Distributed kNN retrieval kernel for Trainium2 (8 NeuronCores).

Computes, for query batch B=256 against three memory banks of N=131072 rows
(D=512): combined = (0.4*cos(q,Mq) + 0.4*cos(q,Mr) + 0.2*cos(q,Mt)) * strength,
masked below 0.3 to -1.0, then top-5 values + indices per query row
(ties broken by the lowest index, matching jax.lax.top_k).

By linearity of the dot product, the three cosine similarities collapse into
a single effective memory bank computed during host-side sharding (the same
stage that casts to bf16):

    E[n] = strength[n] * (0.4*Mq[n]/|Mq[n]| + 0.4*Mr[n]/|Mr[n]| + 0.2*Mt[n]/|Mt[n]|)
    combined[b, n] = q_hat[b] . E[n]

E is sharded along N across the 8 cores and shipped bf16 in matmul layout
([chunk, d_in_block, k_block, n] so each 512-row chunk is one contiguous
512 KB DMA). The query is normalized + transposed on host as well.

Each core then:
  1. per 512-row chunk: 4 accumulating PE matmuls per query half
     (q_hatT[128,128] @ E_T[128,512]) into f32 PSUM,
  2. drains PSUM with relu(S - 0.3) -> bf16 on the Scalar engine into the
     HIGH u16 half of a [128, 2048] f32 score buffer whose LOW u16 halves
     hold a descending index code (2047 - j), pre-initialized once.  Each
     32-bit word then reads, as an f32, (score, tie-break toward the lower
     index) — exactly jax.lax.top_k's ordering, since all scores are >= 0.
  3. after each 2048-column segment, a single DVE MAX8 over the f32 view
     returns the packed top-8 (value AND index) per row — no FIND_INDEX8
     second pass, and exact duplicate-value handling for free.
Host glue decodes the packed candidates of 8 cores x 8 segments and
reduces to the global top-5 (value desc, index asc) — the standard
distributed-kNN merge.
"""

import sys

if "/opt/trn_rl_repo" not in sys.path:
    sys.path.insert(0, "/opt/trn_rl_repo")

import numpy as np

B = 256
D = 512
N_CORES = 8
CH = 512          # memory rows per chunk (matmul moving free dim)
SEG_CH = 4        # chunks per extraction segment
W = SEG_CH * CH   # segment width in scores
K_OUT = 5
THRESH = 0.3
EPS = 1e-8
WEIGHTS = (0.4, 0.4, 0.2)
N_WARM = 5        # dummy matmuls to lift the PE HAM throttle during DMA ramp
DVE_DRAIN_CPS = (4, 8, 12)  # chunk-pairs whose half-1 drain runs on the DVE

USE_FP8 = True    # fp8e4m3 memories + DoubleRow matmul (2 K-rows/cell/cycle)
SCALE_E = 256.0   # fp8 pre-scales: keep elements in the e4m3 normal range
SCALE_Q = 64.0    # (|E| <= ~0.6 -> <=154, |q_hat| <= 1 -> <=64; both < 240)

_cache = {}


def _build(ns, split_waits=True):
    """Build the per-core Bass program for a shard of ns memory rows."""
    import concourse.bass as bass
    import concourse.mybir as mybir
    from concourse.tile import TileContext
    from contextlib import ExitStack

    f32 = mybir.dt.float32
    bf16 = mybir.dt.bfloat16
    u32 = mybir.dt.uint32
    Act = mybir.ActivationFunctionType

    mdt = mybir.dt.float8e4 if USE_FP8 else bf16
    # The drain stores relu(S' - thr') where S' is the device-scaled score
    # and thr' = 0.3 * scale: identical bf16 mantissa as relu(S - 0.3),
    # exponent-shifted, so the packed ordering is unchanged; the host
    # divides the scale back out.
    thr_dev = THRESH * (SCALE_E * SCALE_Q if USE_FP8 else 1.0)

    n_chunks = ns // CH
    n_segs = n_chunks // SEG_CH

    nc = bass.Bass(trn_type="TRN2")

    q_d = nc.dram_tensor("q", [128, 2, 4, 128], mdt, kind="ExternalInput")
    e_d = nc.dram_tensor("e", [n_chunks * 128, 4, CH], mdt,
                         kind="ExternalInput")
    codes_d = nc.dram_tensor("codes", [128, W], u32, kind="ExternalInput")
    vals_d = nc.dram_tensor("vals8", [B, n_segs * 8], f32,
                            kind="ExternalOutput")

    q_ap = q_d.ap()
    e_ap = e_d.ap()
    vals_ap = vals_d.ap()

    with TileContext(nc) as tc, ExitStack() as ctx:
        consts = ctx.enter_context(tc.tile_pool(name="consts", bufs=1))
        mpool = ctx.enter_context(tc.tile_pool(name="mpool", bufs=4))
        psum = ctx.enter_context(tc.tile_pool(name="psum", bufs=3, space="PSUM"))
        psum_w = ctx.enter_context(tc.tile_pool(name="psum_w", bufs=1,
                                                space="PSUM"))

        # PE pre-warm: ~4us of dummy matmuls issued while the first real
        # chunk DMAs land, so the HAM clock gate reaches K=8/8 before the
        # main loop starts (else the first ~3.4us of real matmuls run at
        # half clock).
        scratch = consts.tile([128, CH], mdt)
        nc.vector.memset(scratch, 0.0)
        ps_warm = psum_w.tile([128, CH], f32)
        for _ in range(N_WARM):
            nc.tensor.matmul(ps_warm, scratch[:, :128], scratch,
                             start=True, stop=True)

        # Pre-normalized, pre-transposed query: [d_in_block, half, kblk, b].
        qT = consts.tile([128, 2, 4, 128], mdt)
        nc.sync.dma_start(qT, q_ap)

        nthr = consts.tile([128, 1], f32)
        nc.vector.memset(nthr, -thr_dev)

        # Score buffers: [128, W] f32, double-buffered per half.  Low u16 of
        # each word = index code (2047 - j), loaded once; high u16 = bf16
        # score, rewritten by every drain pass.
        r00 = consts.tile([128, W], f32, tag="r00")
        r01 = consts.tile([128, W], f32, tag="r01")
        r10 = consts.tile([128, W], f32, tag="r10")
        r11 = consts.tile([128, W], f32, tag="r11")
        row32 = [[r00, r01], [r10, r11]]  # [set][half]
        # Stride-2 bf16 views of the score halves (little-endian: the high
        # u16 of word j is element 2j+1 of the bf16 view).
        hi = [[t[:, :].bitcast(bf16).rearrange("p (j two) -> p j two", two=2)
               for t in pair] for pair in row32]

        # Packed top-8 per segment per half, accumulated then shipped once.
        pc0 = consts.tile([128, n_segs * 8], f32, tag="pc0")
        pc1 = consts.tile([128, n_segs * 8], f32, tag="pc1")
        pcand = [pc0, pc1]

        for cp in range(n_chunks // 2):
            c0 = cp * 2
            ets = []
            for ci in range(2):
                c = c0 + ci
                et = mpool.tile([128, 4, CH], mdt, tag=f"et{ci}")
                nc.sync.dma_start(et, e_ap[c * 128:(c + 1) * 128])
                ets.append(et)

            if cp in (1, 3):
                # Index codes for the packed score words — two half-size DMAs
                # slipped between chunk DMAs so neither starves the matmul
                # prefetch (codes are only needed before the first MAX8, 4
                # chunks in; the copies run in the DVE's early idle window).
                hw = W // 2
                lo = 0 if cp == 1 else hw
                nc.sync.dma_start(
                    r00[:, lo:lo + hw].bitcast(u32), codes_d.ap()[:, lo:lo + hw])
                if cp == 3:
                    for t in (r01, r10, r11):
                        nc.vector.tensor_copy(
                            t[:, :].bitcast(u32), r00[:, :].bitcast(u32))

            s = c0 // SEG_CH
            cq = c0 % SEG_CH
            for half in range(2):
                # Two chunks' scores into two adjacent PSUM banks, drained
                # by a single ACT pass (halves the per-op overhead).
                ps = psum.tile([128, 2, CH], f32, tag="S")
                for ci in range(2):
                    if USE_FP8:
                        for kh in range(2):
                            nc.tensor.matmul(
                                ps[:, ci, :],
                                qT[:, half, 2 * kh:2 * kh + 2, :],
                                ets[ci][:, 2 * kh:2 * kh + 2, :],
                                start=(kh == 0), stop=(kh == 1),
                                perf_mode=mybir.MatmulPerfMode.DoubleRow,
                            )
                    else:
                        for kb in range(4):
                            nc.tensor.matmul(
                                ps[:, ci, :], qT[:, half, kb, :],
                                ets[ci][:, kb, :],
                                start=(kb == 0), stop=(kb == 3),
                            )
                # relu(S' - thr') straight into the packed buffer's score
                # slots.  Masked entries become 0; survivors keep their
                # (shifted, device-scaled) score, order preserved.  Most
                # drains run on ACT; a few run on the DVE's spare capacity
                # to balance the two post-matmul engines.
                out_slots = hi[s % 2][half][:, cq * CH:(cq + 2) * CH, 1]
                ps_flat = ps.rearrange("p two n -> p (two n)")
                if cp in DVE_DRAIN_CPS and half == 1:
                    nc.vector.tensor_scalar(
                        out_slots, ps_flat, -thr_dev, 0.0,
                        op0=mybir.AluOpType.add, op1=mybir.AluOpType.max)
                else:
                    nc.scalar.activation(
                        out_slots, ps_flat, Act.Relu, bias=nthr)

            if (c0 + 2) % SEG_CH == 0:
                for half in range(2):
                    nc.vector.max(
                        out=pcand[half][:, s * 8:(s + 1) * 8],
                        in_=row32[s % 2][half])

        for half in range(2):
            nc.sync.dma_start(
                vals_ap[half * 128:(half + 1) * 128, :], pcand[half])

    if split_waits:
        _split_tsp_waits(nc, mybir)
    return nc


def _split_tsp_waits(nc, mybir):
    """This walrus build rejects ANY instruction carrying more than one
    sync-wait command in its encoding (TensorScalarPtr at birverifier;
    LdWeights/Matmult/DMACopy at codegen's setupSyncWait — verified
    empirically: trimming every instruction to one wait compiles). Hoist
    excess waits onto same-engine NoOps inserted just before — engines
    execute their stream in order, so gating the NoOp gates the op. The
    emitted stream order is a valid topological order of Tile's dependency
    graph, so blocking the issuing sequencer on a hoisted wait cannot
    deadlock."""
    skip = {"NoOp"}
    fn = nc.m.functions[0]
    for blk in fn.blocks:
        insts = list(blk.instructions)
        new_insts = []
        changed = False
        for ins in insts:
            si = ins.sync_info
            waits = list(si.on_wait) if si is not None and si.on_wait else []
            if ins.opcode not in skip and len(waits) > 1:
                for wi, w in enumerate(waits[:-1]):
                    new_insts.append(mybir.InstNoOp(
                        name=f"{ins.name}-wn{wi}",
                        engine=ins.engine,
                        sync_info=mybir.SyncInfo(on_wait=[w], on_update=[]),
                    ))
                ins.sync_info = mybir.SyncInfo(
                    on_wait=waits[-1:],
                    on_update=list(si.on_update) if si.on_update else [],
                )
                changed = True
            new_insts.append(ins)
        if changed:
            blk.instructions = new_insts


def _get_program(ns):
    if ns not in _cache:
        _cache[ns] = _build(ns)
    return _cache[ns]


def make_in_maps(query, mem_questions, mem_responses, mem_traces, mem_strengths):
    """Host-side sharding: fold the per-row normalization, bank weights and
    strengths into one effective bf16 memory bank, pre-transposed into
    matmul layout; normalize + transpose the query."""
    import ml_dtypes

    q = np.asarray(query, dtype=np.float32)
    s = np.asarray(mem_strengths, dtype=np.float32)

    if USE_FP8:
        mdt = ml_dtypes.float8_e4m3
        qs, es = SCALE_Q, SCALE_E
    else:
        mdt = ml_dtypes.bfloat16
        qs, es = 1.0, 1.0

    qh = q / (np.linalg.norm(q, axis=1, keepdims=True) + EPS)
    # [p, half, kb, b] = qh[half*128 + b, kb*128 + p]
    qT = np.ascontiguousarray(
        qh.reshape(2, 128, 4, 128).transpose(3, 0, 2, 1) * qs
    ).astype(mdt)

    e = None
    for w, m in zip(WEIGHTS,
                    (mem_questions, mem_responses, mem_traces)):
        m = np.asarray(m, dtype=np.float32)
        f = (w / (np.sqrt(np.einsum('nd,nd->n', m, m)) + EPS)).astype(
            np.float32)
        t = m * f[:, None]
        e = t if e is None else e + t
    e *= s[:, None] * es
    if USE_FP8:
        np.clip(e, -240.0, 240.0, out=e)
    e16 = e.astype(mdt)

    codes = np.broadcast_to(
        np.arange(W - 1, -1, -1, dtype=np.uint32)[None, :], (128, W)
    ).copy()

    n = e16.shape[0]
    ns = n // N_CORES
    n_chunks = ns // CH
    in_maps = []
    for c in range(N_CORES):
        ec = e16[c * ns:(c + 1) * ns]
        # [chunk*128 + p, kb, n] = ec[chunk*CH + n, kb*128 + p]
        ed = np.ascontiguousarray(
            ec.reshape(n_chunks, CH, 4, 128).transpose(0, 3, 2, 1)
        ).reshape(n_chunks * 128, 4, CH)
        in_maps.append({"q": qT, "e": ed, "codes": codes})
    return in_maps, ns


def merge_candidates(per_core, ns, k):
    """Decode the packed (bf16 score | index code) candidates of all cores
    and segments, apply the 0.3 threshold mask, and reduce to the global
    top-k (value desc, global index asc) — matching jax.lax.top_k on the
    masked array.

    Exactness of the -1 fills: a fill slot only occurs when fewer than k
    values globally exceed the threshold, in which case every survivor is
    within its segment's top-8, so the survivor set is complete; the -1
    entries of the reference's top-k are then the smallest global indices
    not occupied by survivors (all masked entries tie at -1; top_k breaks
    ties by the lowest index)."""
    import ml_dtypes

    inv = 1.0 / (SCALE_E * SCALE_Q) if USE_FP8 else 1.0
    n_segs = ns // W
    packed = np.concatenate(
        [np.ascontiguousarray(np.asarray(r["vals8"], dtype=np.float32))
         for r in per_core], axis=1)           # [B, n_cores * n_segs * 8]
    bits = packed.view(np.uint32)
    cand_vals = (bits >> 16).astype(np.uint16).view(
        ml_dtypes.bfloat16).astype(np.float32) * inv
    j_local = (W - 1) - (bits & 0xFFFF).astype(np.int64)
    seg = np.tile(np.repeat(np.arange(n_segs), 8)[None, :], (1, len(per_core)))
    core = np.repeat(np.arange(len(per_core)), n_segs * 8)[None, :]
    cand_idx = core * ns + seg * W + j_local

    # Device ships relu(S - 0.3) (device-scaled): survivors are > 0;
    # shift back to S.
    surv = cand_vals > 0.0
    masked_vals = np.where(surv, cand_vals + THRESH, -np.inf)
    order1 = np.argsort(cand_idx, axis=1, kind="stable")
    v1 = np.take_along_axis(masked_vals, order1, axis=1)
    i1 = np.take_along_axis(cand_idx, order1, axis=1)
    order2 = np.argsort(-v1, axis=1, kind="stable")
    vals = np.take_along_axis(v1, order2, axis=1)[:, :k].copy()
    idx = np.take_along_axis(i1, order2, axis=1)[:, :k].copy()
    # Fill non-survivor slots with (-1.0, smallest free global indices).
    nrows = vals.shape[0]
    for r in range(nrows):
        m = int((vals[r] > -np.inf).sum())
        if m >= k:
            continue
        taken = set(int(x) for x in idx[r, :m])
        fill = []
        cand = 0
        while len(fill) < k - m:
            if cand not in taken:
                fill.append(cand)
            cand += 1
        vals[r, m:] = -1.0
        idx[r, m:] = fill
    return vals.astype(np.float32), idx.astype(np.int32)


def _install_ntff_shim():
    """Register the axon NTFF profile hook (the agent image lacks
    antenv.axon_hooks; recreate it per the documented ctypes C ABI)."""
    import sys as _sys
    import types
    import ctypes
    import contextlib

    if "antenv.axon_hooks" in _sys.modules:
        return
    so_path = "/opt/axon/libaxon_pjrt.so"
    lib = ctypes.CDLL(so_path)
    if not hasattr(lib, "axon_start_nrt_profile"):
        return
    lib.axon_start_nrt_profile.argtypes = [
        ctypes.POINTER(ctypes.c_int64), ctypes.c_size_t]
    lib.axon_start_nrt_profile.restype = ctypes.c_int64
    lib.axon_stop_nrt_profile.argtypes = [ctypes.c_char_p]
    lib.axon_stop_nrt_profile.restype = ctypes.c_int64

    @contextlib.contextmanager
    def _hook(output_dir, device_ids):
        import jax
        jax.devices()
        if device_ids:
            ids = (ctypes.c_int64 * len(device_ids))(*device_ids)
            rc = lib.axon_start_nrt_profile(ids, len(device_ids))
        else:
            rc = lib.axon_start_nrt_profile(None, 0)
        if rc != 0:
            raise RuntimeError(f"axon_start_nrt_profile rc={rc}")
        try:
            yield
        finally:
            n = lib.axon_stop_nrt_profile(str(output_dir).encode())
            print(f"ntff profile: {n} file(s) written to {output_dir}",
                  file=_sys.stderr)

    mod = types.ModuleType("antenv.axon_hooks")
    mod._hook = _hook
    mod.get_axon_ntff_profile_hook = lambda: _hook
    mod.set_axon_ntff_profile_hook = lambda h: None
    _sys.modules["antenv.axon_hooks"] = mod


def kernel(query, mem_questions, mem_responses, mem_traces, mem_strengths,
           top_k, _trace=False, _results_box=None):
    from concourse import bass_utils

    if _trace:
        _install_ntff_shim()

    k = int(top_k)
    in_maps, ns = make_in_maps(
        query, mem_questions, mem_responses, mem_traces, mem_strengths)
    nc = _get_program(ns)
    res = bass_utils.run_bass_kernel_spmd(
        nc, in_maps, core_ids=list(range(N_CORES)), trace=_trace)
    if _results_box is not None:
        _results_box.append(res)
    return merge_candidates(res.results, ns, k)


# revision 28
# speedup vs baseline: 1.1015x; 1.1015x over previous
"""Distributed kNN retrieval kernel for Trainium2 (8 NeuronCores).

Computes, for query batch B=256 against three memory banks of N=131072 rows
(D=512): combined = (0.4*cos(q,Mq) + 0.4*cos(q,Mr) + 0.2*cos(q,Mt)) * strength,
masked below 0.3 to -1.0, then top-5 values + indices per query row
(ties broken by the lowest index, matching jax.lax.top_k).

By linearity of the dot product, the three cosine similarities collapse into
a single effective memory bank computed during host-side sharding (the same
stage that casts to bf16):

    E[n] = strength[n] * (0.4*Mq[n]/|Mq[n]| + 0.4*Mr[n]/|Mr[n]| + 0.2*Mt[n]/|Mt[n]|)
    combined[b, n] = q_hat[b] . E[n]

E is sharded along N across the 8 cores and shipped bf16 in matmul layout
([chunk, d_in_block, k_block, n] so each 512-row chunk is one contiguous
512 KB DMA). The query is normalized + transposed on host as well.

Each core then:
  1. per 512-row chunk: 4 accumulating PE matmuls per query half
     (q_hatT[128,128] @ E_T[128,512]) into f32 PSUM,
  2. drains PSUM with relu(S - 0.3) -> bf16 on the Scalar engine into the
     HIGH u16 half of a [128, 2048] f32 score buffer whose LOW u16 halves
     hold a descending index code (2047 - j), pre-initialized once.  Each
     32-bit word then reads, as an f32, (score, tie-break toward the lower
     index) — exactly jax.lax.top_k's ordering, since all scores are >= 0.
  3. after each 2048-column segment, a single DVE MAX8 over the f32 view
     returns the packed top-8 (value AND index) per row — no FIND_INDEX8
     second pass, and exact duplicate-value handling for free.
Host glue decodes the packed candidates of 8 cores x 8 segments and
reduces to the global top-5 (value desc, index asc) — the standard
distributed-kNN merge.
"""

import sys

if "/opt/trn_rl_repo" not in sys.path:
    sys.path.insert(0, "/opt/trn_rl_repo")

import numpy as np

B = 256
D = 512
N_CORES = 8
CH = 512          # memory rows per chunk (matmul moving free dim)
SEG_CH = 4        # chunks per extraction segment
W = SEG_CH * CH   # segment width in scores
K_OUT = 5
THRESH = 0.3
EPS = 1e-8
WEIGHTS = (0.4, 0.4, 0.2)
N_WARM = 10       # dummy matmuls to lift the PE HAM throttle during DMA ramp
DVE_DRAIN_CPS = (4, 8, 12)  # chunk-pairs whose half-1 drain runs on the DVE

USE_FP8 = True    # fp8e4m3 memories + DoubleRow matmul (2 K-rows/cell/cycle)
SCALE_E = 256.0   # fp8 pre-scales: keep elements in the e4m3 normal range
SCALE_Q = 64.0    # (|E| <= ~0.6 -> <=154, |q_hat| <= 1 -> <=64; both < 240)

_cache = {}


def _build(ns, split_waits=True):
    """Build the per-core Bass program for a shard of ns memory rows."""
    import concourse.bass as bass
    import concourse.mybir as mybir
    from concourse.tile import TileContext
    from contextlib import ExitStack

    f32 = mybir.dt.float32
    bf16 = mybir.dt.bfloat16
    u32 = mybir.dt.uint32
    Act = mybir.ActivationFunctionType

    mdt = mybir.dt.float8e4 if USE_FP8 else bf16
    # The drain stores relu(S' - thr') where S' is the device-scaled score
    # and thr' = 0.3 * scale: identical bf16 mantissa as relu(S - 0.3),
    # exponent-shifted, so the packed ordering is unchanged; the host
    # divides the scale back out.
    thr_dev = THRESH * (SCALE_E * SCALE_Q if USE_FP8 else 1.0)

    n_chunks = ns // CH
    n_segs = n_chunks // SEG_CH

    nc = bass.Bass(trn_type="TRN2")

    q_d = nc.dram_tensor("q", [128, 2, 4, 128], mdt, kind="ExternalInput")
    e_d = nc.dram_tensor("e", [n_chunks * 128, 4, CH], mdt,
                         kind="ExternalInput")
    codes_d = nc.dram_tensor("codes", [128, W], u32, kind="ExternalInput")
    vals_d = nc.dram_tensor("vals8", [B, n_segs * 8], f32,
                            kind="ExternalOutput")

    q_ap = q_d.ap()
    e_ap = e_d.ap()
    vals_ap = vals_d.ap()

    with TileContext(nc) as tc, ExitStack() as ctx:
        consts = ctx.enter_context(tc.tile_pool(name="consts", bufs=1))
        mpool = ctx.enter_context(tc.tile_pool(name="mpool", bufs=6))
        psum = ctx.enter_context(tc.tile_pool(name="psum", bufs=4, space="PSUM"))

        # PE pre-warm: ~3us of dummy matmuls issued while the first real
        # chunk DMAs land, so the HAM clock gate reaches K=8/8 soon after the
        # main loop starts (else the first ~3.4us of real matmuls run at
        # half clock).  The warm tile is generation 0 of the regular PSUM
        # rotation — its banks recycle into the main loop.
        scratch = consts.tile([128, CH], mdt)
        nc.vector.memset(scratch, 0.0)
        ps_warm = psum.tile([128, 2, CH], f32, tag="S")
        for _ in range(N_WARM):
            nc.tensor.matmul(ps_warm[:, 0, :], scratch[:, :128], scratch,
                             start=True, stop=True)

        # Pre-normalized, pre-transposed query: [d_in_block, half, kblk, b].
        qT = consts.tile([128, 2, 4, 128], mdt)
        nc.sync.dma_start(qT, q_ap)

        nthr = consts.tile([128, 1], f32)
        nc.vector.memset(nthr, -thr_dev)

        # Score buffers: [128, W] f32, double-buffered per half.  Low u16 of
        # each word = index code (2047 - j), loaded once; high u16 = bf16
        # score, rewritten by every drain pass.
        r00 = consts.tile([128, W], f32, tag="r00")
        r01 = consts.tile([128, W], f32, tag="r01")
        r10 = consts.tile([128, W], f32, tag="r10")
        r11 = consts.tile([128, W], f32, tag="r11")
        row32 = [[r00, r01], [r10, r11]]  # [set][half]
        # Index codes: DMA'd ahead of the matmul operands (the 10 warm
        # matmuls bridge the extra ~3us of DMA ramp), then fanned out to the
        # other buffers by the DVE during its initial idle window — all
        # before any drain writes score slots (program order keeps this
        # safe).
        nc.sync.dma_start(r00[:, :].bitcast(u32), codes_d.ap())
        for t in (r01, r10, r11):
            nc.vector.tensor_copy(
                t[:, :].bitcast(u32), r00[:, :].bitcast(u32))
        # Stride-2 bf16 views of the score halves (little-endian: the high
        # u16 of word j is element 2j+1 of the bf16 view).
        hi = [[t[:, :].bitcast(bf16).rearrange("p (j two) -> p j two", two=2)
               for t in pair] for pair in row32]

        # Packed top-8 per segment per half, accumulated then shipped once.
        pc0 = consts.tile([128, n_segs * 8], f32, tag="pc0")
        pc1 = consts.tile([128, n_segs * 8], f32, tag="pc1")
        pcand = [pc0, pc1]

        for cp in range(n_chunks // 2):
            c0 = cp * 2
            ets = []
            for ci in range(2):
                c = c0 + ci
                et = mpool.tile([128, 4, CH], mdt, tag=f"et{ci}")
                nc.sync.dma_start(et, e_ap[c * 128:(c + 1) * 128])
                ets.append(et)

            s = c0 // SEG_CH
            cq = c0 % SEG_CH
            for half in range(2):
                # Two chunks' scores into two adjacent PSUM banks, drained
                # by a single ACT pass (halves the per-op overhead).
                ps = psum.tile([128, 2, CH], f32, tag="S")
                for ci in range(2):
                    if USE_FP8:
                        for kh in range(2):
                            nc.tensor.matmul(
                                ps[:, ci, :],
                                qT[:, half, 2 * kh:2 * kh + 2, :],
                                ets[ci][:, 2 * kh:2 * kh + 2, :],
                                start=(kh == 0), stop=(kh == 1),
                                perf_mode=mybir.MatmulPerfMode.DoubleRow,
                            )
                    else:
                        for kb in range(4):
                            nc.tensor.matmul(
                                ps[:, ci, :], qT[:, half, kb, :],
                                ets[ci][:, kb, :],
                                start=(kb == 0), stop=(kb == 3),
                            )
                # relu(S' - thr') straight into the packed buffer's score
                # slots.  Masked entries become 0; survivors keep their
                # (shifted, device-scaled) score, order preserved.  Most
                # drains run on ACT; a few run on the DVE's spare capacity
                # to balance the two post-matmul engines.
                out_slots = hi[s % 2][half][:, cq * CH:(cq + 2) * CH, 1]
                ps_flat = ps.rearrange("p two n -> p (two n)")
                if cp in DVE_DRAIN_CPS and half == 1:
                    nc.vector.tensor_scalar(
                        out_slots, ps_flat, -thr_dev, 0.0,
                        op0=mybir.AluOpType.add, op1=mybir.AluOpType.max)
                else:
                    nc.scalar.activation(
                        out_slots, ps_flat, Act.Relu, bias=nthr)

            if (c0 + 2) % SEG_CH == 0:
                for half in range(2):
                    nc.vector.max(
                        out=pcand[half][:, s * 8:(s + 1) * 8],
                        in_=row32[s % 2][half])

        for half in range(2):
            nc.sync.dma_start(
                vals_ap[half * 128:(half + 1) * 128, :], pcand[half])

    if split_waits:
        _split_tsp_waits(nc, mybir)
    return nc


def _split_tsp_waits(nc, mybir):
    """This walrus build rejects ANY instruction carrying more than one
    sync-wait command in its encoding (TensorScalarPtr at birverifier;
    LdWeights/Matmult/DMACopy at codegen's setupSyncWait — verified
    empirically: trimming every instruction to one wait compiles). Hoist
    excess waits onto same-engine NoOps inserted just before — engines
    execute their stream in order, so gating the NoOp gates the op. The
    emitted stream order is a valid topological order of Tile's dependency
    graph, so blocking the issuing sequencer on a hoisted wait cannot
    deadlock."""
    skip = {"NoOp"}
    fn = nc.m.functions[0]
    for blk in fn.blocks:
        insts = list(blk.instructions)
        new_insts = []
        changed = False
        for ins in insts:
            si = ins.sync_info
            waits = list(si.on_wait) if si is not None and si.on_wait else []
            if ins.opcode not in skip and len(waits) > 1:
                for wi, w in enumerate(waits[:-1]):
                    new_insts.append(mybir.InstNoOp(
                        name=f"{ins.name}-wn{wi}",
                        engine=ins.engine,
                        sync_info=mybir.SyncInfo(on_wait=[w], on_update=[]),
                    ))
                ins.sync_info = mybir.SyncInfo(
                    on_wait=waits[-1:],
                    on_update=list(si.on_update) if si.on_update else [],
                )
                changed = True
            new_insts.append(ins)
        if changed:
            blk.instructions = new_insts


def _get_program(ns):
    if ns not in _cache:
        _cache[ns] = _build(ns)
    return _cache[ns]


def make_in_maps(query, mem_questions, mem_responses, mem_traces, mem_strengths):
    """Host-side sharding: fold the per-row normalization, bank weights and
    strengths into one effective bf16 memory bank, pre-transposed into
    matmul layout; normalize + transpose the query."""
    import ml_dtypes

    q = np.asarray(query, dtype=np.float32)
    s = np.asarray(mem_strengths, dtype=np.float32)

    if USE_FP8:
        mdt = ml_dtypes.float8_e4m3
        qs, es = SCALE_Q, SCALE_E
    else:
        mdt = ml_dtypes.bfloat16
        qs, es = 1.0, 1.0

    qh = q / (np.linalg.norm(q, axis=1, keepdims=True) + EPS)
    # [p, half, kb, b] = qh[half*128 + b, kb*128 + p]
    qT = np.ascontiguousarray(
        qh.reshape(2, 128, 4, 128).transpose(3, 0, 2, 1) * qs
    ).astype(mdt)

    e = None
    for w, m in zip(WEIGHTS,
                    (mem_questions, mem_responses, mem_traces)):
        m = np.asarray(m, dtype=np.float32)
        f = (w / (np.sqrt(np.einsum('nd,nd->n', m, m)) + EPS)).astype(
            np.float32)
        t = m * f[:, None]
        e = t if e is None else e + t
    e *= s[:, None] * es
    if USE_FP8:
        np.clip(e, -240.0, 240.0, out=e)
    e16 = e.astype(mdt)

    codes = np.broadcast_to(
        np.arange(W - 1, -1, -1, dtype=np.uint32)[None, :], (128, W)
    ).copy()

    n = e16.shape[0]
    ns = n // N_CORES
    n_chunks = ns // CH
    in_maps = []
    for c in range(N_CORES):
        ec = e16[c * ns:(c + 1) * ns]
        # [chunk*128 + p, kb, n] = ec[chunk*CH + n, kb*128 + p]
        ed = np.ascontiguousarray(
            ec.reshape(n_chunks, CH, 4, 128).transpose(0, 3, 2, 1)
        ).reshape(n_chunks * 128, 4, CH)
        in_maps.append({"q": qT, "e": ed, "codes": codes})
    return in_maps, ns


def merge_candidates(per_core, ns, k):
    """Decode the packed (bf16 score | index code) candidates of all cores
    and segments, apply the 0.3 threshold mask, and reduce to the global
    top-k (value desc, global index asc) — matching jax.lax.top_k on the
    masked array.

    Exactness of the -1 fills: a fill slot only occurs when fewer than k
    values globally exceed the threshold, in which case every survivor is
    within its segment's top-8, so the survivor set is complete; the -1
    entries of the reference's top-k are then the smallest global indices
    not occupied by survivors (all masked entries tie at -1; top_k breaks
    ties by the lowest index)."""
    import ml_dtypes

    inv = 1.0 / (SCALE_E * SCALE_Q) if USE_FP8 else 1.0
    n_segs = ns // W
    packed = np.concatenate(
        [np.ascontiguousarray(np.asarray(r["vals8"], dtype=np.float32))
         for r in per_core], axis=1)           # [B, n_cores * n_segs * 8]
    bits = packed.view(np.uint32)
    cand_vals = (bits >> 16).astype(np.uint16).view(
        ml_dtypes.bfloat16).astype(np.float32) * inv
    j_local = (W - 1) - (bits & 0xFFFF).astype(np.int64)
    seg = np.tile(np.repeat(np.arange(n_segs), 8)[None, :], (1, len(per_core)))
    core = np.repeat(np.arange(len(per_core)), n_segs * 8)[None, :]
    cand_idx = core * ns + seg * W + j_local

    # Device ships relu(S - 0.3) (device-scaled): survivors are > 0;
    # shift back to S.
    surv = cand_vals > 0.0
    masked_vals = np.where(surv, cand_vals + THRESH, -np.inf)
    order1 = np.argsort(cand_idx, axis=1, kind="stable")
    v1 = np.take_along_axis(masked_vals, order1, axis=1)
    i1 = np.take_along_axis(cand_idx, order1, axis=1)
    order2 = np.argsort(-v1, axis=1, kind="stable")
    vals = np.take_along_axis(v1, order2, axis=1)[:, :k].copy()
    idx = np.take_along_axis(i1, order2, axis=1)[:, :k].copy()
    # Fill non-survivor slots with (-1.0, smallest free global indices).
    nrows = vals.shape[0]
    for r in range(nrows):
        m = int((vals[r] > -np.inf).sum())
        if m >= k:
            continue
        taken = set(int(x) for x in idx[r, :m])
        fill = []
        cand = 0
        while len(fill) < k - m:
            if cand not in taken:
                fill.append(cand)
            cand += 1
        vals[r, m:] = -1.0
        idx[r, m:] = fill
    return vals.astype(np.float32), idx.astype(np.int32)


def _install_ntff_shim():
    """Register the axon NTFF profile hook (the agent image lacks
    antenv.axon_hooks; recreate it per the documented ctypes C ABI)."""
    import sys as _sys
    import types
    import ctypes
    import contextlib

    if "antenv.axon_hooks" in _sys.modules:
        return
    so_path = "/opt/axon/libaxon_pjrt.so"
    lib = ctypes.CDLL(so_path)
    if not hasattr(lib, "axon_start_nrt_profile"):
        return
    lib.axon_start_nrt_profile.argtypes = [
        ctypes.POINTER(ctypes.c_int64), ctypes.c_size_t]
    lib.axon_start_nrt_profile.restype = ctypes.c_int64
    lib.axon_stop_nrt_profile.argtypes = [ctypes.c_char_p]
    lib.axon_stop_nrt_profile.restype = ctypes.c_int64

    @contextlib.contextmanager
    def _hook(output_dir, device_ids):
        import jax
        jax.devices()
        if device_ids:
            ids = (ctypes.c_int64 * len(device_ids))(*device_ids)
            rc = lib.axon_start_nrt_profile(ids, len(device_ids))
        else:
            rc = lib.axon_start_nrt_profile(None, 0)
        if rc != 0:
            raise RuntimeError(f"axon_start_nrt_profile rc={rc}")
        try:
            yield
        finally:
            n = lib.axon_stop_nrt_profile(str(output_dir).encode())
            print(f"ntff profile: {n} file(s) written to {output_dir}",
                  file=_sys.stderr)

    mod = types.ModuleType("antenv.axon_hooks")
    mod._hook = _hook
    mod.get_axon_ntff_profile_hook = lambda: _hook
    mod.set_axon_ntff_profile_hook = lambda h: None
    _sys.modules["antenv.axon_hooks"] = mod


def kernel(query, mem_questions, mem_responses, mem_traces, mem_strengths,
           top_k, _trace=False, _results_box=None):
    from concourse import bass_utils

    if _trace:
        _install_ntff_shim()

    k = int(top_k)
    in_maps, ns = make_in_maps(
        query, mem_questions, mem_responses, mem_traces, mem_strengths)
    nc = _get_program(ns)
    res = bass_utils.run_bass_kernel_spmd(
        nc, in_maps, core_ids=list(range(N_CORES)), trace=_trace)
    if _results_box is not None:
        _results_box.append(res)
    return merge_candidates(res.results, ns, k)


# revision 34
# speedup vs baseline: 1.1439x; 1.0384x over previous
"""Distributed kNN retrieval kernel for Trainium2 (8 NeuronCores).

Computes, for query batch B=256 against three memory banks of N=131072 rows
(D=512): combined = (0.4*cos(q,Mq) + 0.4*cos(q,Mr) + 0.2*cos(q,Mt)) * strength,
masked below 0.3 to -1.0, then top-5 values + indices per query row
(ties broken by the lowest index, matching jax.lax.top_k).

By linearity of the dot product, the three cosine similarities collapse into
a single effective memory bank computed during host-side sharding (the same
stage that casts to bf16):

    E[n] = strength[n] * (0.4*Mq[n]/|Mq[n]| + 0.4*Mr[n]/|Mr[n]| + 0.2*Mt[n]/|Mt[n]|)
    combined[b, n] = q_hat[b] . E[n]

E is sharded along N across the 8 cores and shipped bf16 in matmul layout
([chunk, d_in_block, k_block, n] so each 512-row chunk is one contiguous
512 KB DMA). The query is normalized + transposed on host as well.

Each core then:
  1. per 512-row chunk: 4 accumulating PE matmuls per query half
     (q_hatT[128,128] @ E_T[128,512]) into f32 PSUM,
  2. drains PSUM with relu(S - 0.3) -> bf16 on the Scalar engine into the
     HIGH u16 half of a [128, 2048] f32 score buffer whose LOW u16 halves
     hold a descending index code (2047 - j), pre-initialized once.  Each
     32-bit word then reads, as an f32, (score, tie-break toward the lower
     index) — exactly jax.lax.top_k's ordering, since all scores are >= 0.
  3. after each 2048-column segment, a single DVE MAX8 over the f32 view
     returns the packed top-8 (value AND index) per row — no FIND_INDEX8
     second pass, and exact duplicate-value handling for free.
Host glue decodes the packed candidates of 8 cores x 8 segments and
reduces to the global top-5 (value desc, index asc) — the standard
distributed-kNN merge.
"""

import sys

if "/opt/trn_rl_repo" not in sys.path:
    sys.path.insert(0, "/opt/trn_rl_repo")

import numpy as np

B = 256
D = 512
N_CORES = 8
CH = 512          # memory rows per chunk (matmul moving free dim)
SEG_CH = 4        # chunks per extraction segment
W = SEG_CH * CH   # segment width in scores
K_OUT = 5
THRESH = 0.3
EPS = 1e-8
WEIGHTS = (0.4, 0.4, 0.2)
N_WARM = 10       # dummy matmuls to lift the PE HAM throttle during DMA ramp
DVE_DRAIN_CPS = (1, 2)  # chunk-pairs whose half-1 drain runs on the DVE
                        # (early ones only — the DVE idles until the first
                        # MAX8, so these are free; later ones would push the
                        # MAX8 backlog out)

USE_FP8 = True    # fp8e4m3 memories + DoubleRow matmul (2 K-rows/cell/cycle)
SCALE_E = 256.0   # fp8 pre-scales: keep elements in the e4m3 normal range
SCALE_Q = 64.0    # (|E| <= ~0.6 -> <=154, |q_hat| <= 1 -> <=64; both < 240)

_cache = {}


def _build(ns, split_waits=True):
    """Build the per-core Bass program for a shard of ns memory rows."""
    import concourse.bass as bass
    import concourse.mybir as mybir
    from concourse.tile import TileContext
    from contextlib import ExitStack

    f32 = mybir.dt.float32
    bf16 = mybir.dt.bfloat16
    u32 = mybir.dt.uint32
    Act = mybir.ActivationFunctionType

    mdt = mybir.dt.float8e4 if USE_FP8 else bf16
    # The drain stores relu(S' - thr') where S' is the device-scaled score
    # and thr' = 0.3 * scale: identical bf16 mantissa as relu(S - 0.3),
    # exponent-shifted, so the packed ordering is unchanged; the host
    # divides the scale back out.
    thr_dev = THRESH * (SCALE_E * SCALE_Q if USE_FP8 else 1.0)

    n_chunks = ns // CH
    n_segs = n_chunks // SEG_CH

    nc = bass.Bass(trn_type="TRN2")

    q_d = nc.dram_tensor("q", [128, 2, 4, 128], mdt, kind="ExternalInput")
    e_d = nc.dram_tensor("e", [n_chunks * 128, 4, CH], mdt,
                         kind="ExternalInput")
    codes_d = nc.dram_tensor("codes", [128, W], u32, kind="ExternalInput")
    n_units = n_segs + 1   # last segment ships as two half-width units
    vals_d = nc.dram_tensor("vals8", [B, n_units * 8], f32,
                            kind="ExternalOutput")

    q_ap = q_d.ap()
    e_ap = e_d.ap()
    vals_ap = vals_d.ap()

    with TileContext(nc) as tc, ExitStack() as ctx:
        consts = ctx.enter_context(tc.tile_pool(name="consts", bufs=1))
        mpool = ctx.enter_context(tc.tile_pool(name="mpool", bufs=6))
        psum = ctx.enter_context(tc.tile_pool(name="psum", bufs=4, space="PSUM"))

        # PE pre-warm: ~3us of dummy matmuls issued while the first real
        # chunk DMAs land, so the HAM clock gate reaches K=8/8 soon after the
        # main loop starts (else the first ~3.4us of real matmuls run at
        # half clock).  The warm tile is generation 0 of the regular PSUM
        # rotation — its banks recycle into the main loop.
        scratch = consts.tile([128, CH], mdt)
        nc.vector.memset(scratch, 0.0)
        ps_warm = psum.tile([128, 2, CH], f32, tag="S")
        for _ in range(N_WARM):
            nc.tensor.matmul(ps_warm[:, 0, :], scratch[:, :128], scratch,
                             start=True, stop=True)

        # Pre-normalized, pre-transposed query: [d_in_block, half, kblk, b].
        qT = consts.tile([128, 2, 4, 128], mdt)
        nc.sync.dma_start(qT, q_ap)

        nthr = consts.tile([128, 1], f32)
        nc.vector.memset(nthr, -thr_dev)

        # Score buffers: [128, W] f32, double-buffered per half.  Low u16 of
        # each word = index code (2047 - j), loaded once; high u16 = bf16
        # score, rewritten by every drain pass.
        r00 = consts.tile([128, W], f32, tag="r00")
        r01 = consts.tile([128, W], f32, tag="r01")
        r10 = consts.tile([128, W], f32, tag="r10")
        r11 = consts.tile([128, W], f32, tag="r11")
        row32 = [[r00, r01], [r10, r11]]  # [set][half]
        # Index codes: DMA'd ahead of the matmul operands (the 10 warm
        # matmuls bridge the extra ~3us of DMA ramp), then fanned out to the
        # other buffers by the DVE during its initial idle window — all
        # before any drain writes score slots (program order keeps this
        # safe).
        nc.sync.dma_start(r00[:, :].bitcast(u32), codes_d.ap())
        for t in (r01, r10, r11):
            nc.vector.tensor_copy(
                t[:, :].bitcast(u32), r00[:, :].bitcast(u32))
        # Stride-2 bf16 views of the score halves (little-endian: the high
        # u16 of word j is element 2j+1 of the bf16 view).
        hi = [[t[:, :].bitcast(bf16).rearrange("p (j two) -> p j two", two=2)
               for t in pair] for pair in row32]

        # Packed top-8 per segment per half, accumulated then shipped once.
        pc0 = consts.tile([128, n_units * 8], f32, tag="pc0")
        pc1 = consts.tile([128, n_units * 8], f32, tag="pc1")
        pcand = [pc0, pc1]

        for cp in range(n_chunks // 2):
            c0 = cp * 2
            ets = []
            for ci in range(2):
                c = c0 + ci
                et = mpool.tile([128, 4, CH], mdt, tag=f"et{ci}")
                nc.sync.dma_start(et, e_ap[c * 128:(c + 1) * 128])
                ets.append(et)

            s = c0 // SEG_CH
            cq = c0 % SEG_CH
            for half in range(2):
                # Two chunks' scores into two adjacent PSUM banks, drained
                # by a single ACT pass (halves the per-op overhead).
                ps = psum.tile([128, 2, CH], f32, tag="S")
                # kh-outer: consecutive matmuls share the same stationary
                # operand, giving the PE's weight path the best shot at
                # pipelining the (FWL-less) DoubleRow LDWEIGHTS.
                if USE_FP8:
                    for kh in range(2):
                        for ci in range(2):
                            nc.tensor.matmul(
                                ps[:, ci, :],
                                qT[:, half, 2 * kh:2 * kh + 2, :],
                                ets[ci][:, 2 * kh:2 * kh + 2, :],
                                start=(kh == 0), stop=(kh == 1),
                                perf_mode=mybir.MatmulPerfMode.DoubleRow,
                            )
                else:
                    for kb in range(4):
                        for ci in range(2):
                            nc.tensor.matmul(
                                ps[:, ci, :], qT[:, half, kb, :],
                                ets[ci][:, kb, :],
                                start=(kb == 0), stop=(kb == 3),
                            )
                # relu(S' - thr') straight into the packed buffer's score
                # slots.  Masked entries become 0; survivors keep their
                # (shifted, device-scaled) score, order preserved.  Most
                # drains run on ACT; a few run on the DVE's spare capacity
                # to balance the two post-matmul engines.
                out_slots = hi[s % 2][half][:, cq * CH:(cq + 2) * CH, 1]
                ps_flat = ps.rearrange("p two n -> p (two n)")
                if cp in DVE_DRAIN_CPS and half == 1:
                    nc.vector.tensor_scalar(
                        out_slots, ps_flat, -thr_dev, 0.0,
                        op0=mybir.AluOpType.add, op1=mybir.AluOpType.max)
                else:
                    nc.scalar.activation(
                        out_slots, ps_flat, Act.Relu, bias=nthr)

            last_seg = (s == n_segs - 1)
            if last_seg:
                # Final segment: two half-width MAX8 units, each fired as
                # soon as its chunk pair is drained — halves the serial
                # DVE tail after the last drain.
                u = n_segs - 1 + (cp % 2)
                lo = (cp % 2) * (W // 2)
                for half in range(2):
                    nc.vector.max(
                        out=pcand[half][:, u * 8:(u + 1) * 8],
                        in_=row32[s % 2][half][:, lo:lo + W // 2])
            elif (c0 + 2) % SEG_CH == 0:
                for half in range(2):
                    nc.vector.max(
                        out=pcand[half][:, s * 8:(s + 1) * 8],
                        in_=row32[s % 2][half])

        for half in range(2):
            nc.sync.dma_start(
                vals_ap[half * 128:(half + 1) * 128, :], pcand[half])

    if split_waits:
        _split_tsp_waits(nc, mybir)
    return nc


def _split_tsp_waits(nc, mybir):
    """This walrus build rejects ANY instruction carrying more than one
    sync-wait command in its encoding (TensorScalarPtr at birverifier;
    LdWeights/Matmult/DMACopy at codegen's setupSyncWait — verified
    empirically: trimming every instruction to one wait compiles). Hoist
    excess waits onto same-engine NoOps inserted just before — engines
    execute their stream in order, so gating the NoOp gates the op. The
    emitted stream order is a valid topological order of Tile's dependency
    graph, so blocking the issuing sequencer on a hoisted wait cannot
    deadlock."""
    skip = {"NoOp"}
    fn = nc.m.functions[0]
    for blk in fn.blocks:
        insts = list(blk.instructions)
        new_insts = []
        changed = False
        for ins in insts:
            si = ins.sync_info
            waits = list(si.on_wait) if si is not None and si.on_wait else []
            if ins.opcode not in skip and len(waits) > 1:
                for wi, w in enumerate(waits[:-1]):
                    new_insts.append(mybir.InstNoOp(
                        name=f"{ins.name}-wn{wi}",
                        engine=ins.engine,
                        sync_info=mybir.SyncInfo(on_wait=[w], on_update=[]),
                    ))
                ins.sync_info = mybir.SyncInfo(
                    on_wait=waits[-1:],
                    on_update=list(si.on_update) if si.on_update else [],
                )
                changed = True
            new_insts.append(ins)
        if changed:
            blk.instructions = new_insts


def _get_program(ns):
    if ns not in _cache:
        _cache[ns] = _build(ns)
    return _cache[ns]


def make_in_maps(query, mem_questions, mem_responses, mem_traces, mem_strengths):
    """Host-side sharding: fold the per-row normalization, bank weights and
    strengths into one effective bf16 memory bank, pre-transposed into
    matmul layout; normalize + transpose the query."""
    import ml_dtypes

    q = np.asarray(query, dtype=np.float32)
    s = np.asarray(mem_strengths, dtype=np.float32)

    if USE_FP8:
        mdt = ml_dtypes.float8_e4m3
        qs, es = SCALE_Q, SCALE_E
    else:
        mdt = ml_dtypes.bfloat16
        qs, es = 1.0, 1.0

    qh = q / (np.linalg.norm(q, axis=1, keepdims=True) + EPS)
    # [p, half, kb, b] = qh[half*128 + b, kb*128 + p]
    qT = np.ascontiguousarray(
        qh.reshape(2, 128, 4, 128).transpose(3, 0, 2, 1) * qs
    ).astype(mdt)

    e = None
    for w, m in zip(WEIGHTS,
                    (mem_questions, mem_responses, mem_traces)):
        m = np.asarray(m, dtype=np.float32)
        f = (w / (np.sqrt(np.einsum('nd,nd->n', m, m)) + EPS)).astype(
            np.float32)
        t = m * f[:, None]
        e = t if e is None else e + t
    e *= s[:, None] * es
    if USE_FP8:
        np.clip(e, -240.0, 240.0, out=e)
    e16 = e.astype(mdt)

    codes = np.broadcast_to(
        np.arange(W - 1, -1, -1, dtype=np.uint32)[None, :], (128, W)
    ).copy()

    n = e16.shape[0]
    ns = n // N_CORES
    n_chunks = ns // CH
    in_maps = []
    for c in range(N_CORES):
        ec = e16[c * ns:(c + 1) * ns]
        # [chunk*128 + p, kb, n] = ec[chunk*CH + n, kb*128 + p]
        ed = np.ascontiguousarray(
            ec.reshape(n_chunks, CH, 4, 128).transpose(0, 3, 2, 1)
        ).reshape(n_chunks * 128, 4, CH)
        in_maps.append({"q": qT, "e": ed, "codes": codes})
    return in_maps, ns


def merge_candidates(per_core, ns, k):
    """Decode the packed (bf16 score | index code) candidates of all cores
    and segments, apply the 0.3 threshold mask, and reduce to the global
    top-k (value desc, global index asc) — matching jax.lax.top_k on the
    masked array.

    Exactness of the -1 fills: a fill slot only occurs when fewer than k
    values globally exceed the threshold, in which case every survivor is
    within its segment's top-8, so the survivor set is complete; the -1
    entries of the reference's top-k are then the smallest global indices
    not occupied by survivors (all masked entries tie at -1; top_k breaks
    ties by the lowest index)."""
    import ml_dtypes

    inv = 1.0 / (SCALE_E * SCALE_Q) if USE_FP8 else 1.0
    n_segs = ns // W
    # Units 0..n_segs-2 are full segments; the last segment ships as two
    # half-width units sharing its segment base (codes stay buffer-local).
    seg_base = np.concatenate([np.arange(n_segs), [n_segs - 1]])
    packed = np.concatenate(
        [np.ascontiguousarray(np.asarray(r["vals8"], dtype=np.float32))
         for r in per_core], axis=1)           # [B, n_cores * n_units * 8]
    bits = packed.view(np.uint32)
    cand_vals = (bits >> 16).astype(np.uint16).view(
        ml_dtypes.bfloat16).astype(np.float32) * inv
    j_local = (W - 1) - (bits & 0xFFFF).astype(np.int64)
    seg = np.tile(np.repeat(seg_base, 8)[None, :], (1, len(per_core)))
    core = np.repeat(np.arange(len(per_core)), len(seg_base) * 8)[None, :]
    cand_idx = core * ns + seg * W + j_local

    # Device ships relu(S - 0.3) (device-scaled): survivors are > 0;
    # shift back to S.
    surv = cand_vals > 0.0
    masked_vals = np.where(surv, cand_vals + THRESH, -np.inf)
    order1 = np.argsort(cand_idx, axis=1, kind="stable")
    v1 = np.take_along_axis(masked_vals, order1, axis=1)
    i1 = np.take_along_axis(cand_idx, order1, axis=1)
    order2 = np.argsort(-v1, axis=1, kind="stable")
    vals = np.take_along_axis(v1, order2, axis=1)[:, :k].copy()
    idx = np.take_along_axis(i1, order2, axis=1)[:, :k].copy()
    # Fill non-survivor slots with (-1.0, smallest free global indices).
    nrows = vals.shape[0]
    for r in range(nrows):
        m = int((vals[r] > -np.inf).sum())
        if m >= k:
            continue
        taken = set(int(x) for x in idx[r, :m])
        fill = []
        cand = 0
        while len(fill) < k - m:
            if cand not in taken:
                fill.append(cand)
            cand += 1
        vals[r, m:] = -1.0
        idx[r, m:] = fill
    return vals.astype(np.float32), idx.astype(np.int32)


def _install_ntff_shim():
    """Register the axon NTFF profile hook (the agent image lacks
    antenv.axon_hooks; recreate it per the documented ctypes C ABI)."""
    import sys as _sys
    import types
    import ctypes
    import contextlib

    if "antenv.axon_hooks" in _sys.modules:
        return
    so_path = "/opt/axon/libaxon_pjrt.so"
    lib = ctypes.CDLL(so_path)
    if not hasattr(lib, "axon_start_nrt_profile"):
        return
    lib.axon_start_nrt_profile.argtypes = [
        ctypes.POINTER(ctypes.c_int64), ctypes.c_size_t]
    lib.axon_start_nrt_profile.restype = ctypes.c_int64
    lib.axon_stop_nrt_profile.argtypes = [ctypes.c_char_p]
    lib.axon_stop_nrt_profile.restype = ctypes.c_int64

    @contextlib.contextmanager
    def _hook(output_dir, device_ids):
        import jax
        jax.devices()
        if device_ids:
            ids = (ctypes.c_int64 * len(device_ids))(*device_ids)
            rc = lib.axon_start_nrt_profile(ids, len(device_ids))
        else:
            rc = lib.axon_start_nrt_profile(None, 0)
        if rc != 0:
            raise RuntimeError(f"axon_start_nrt_profile rc={rc}")
        try:
            yield
        finally:
            n = lib.axon_stop_nrt_profile(str(output_dir).encode())
            print(f"ntff profile: {n} file(s) written to {output_dir}",
                  file=_sys.stderr)

    mod = types.ModuleType("antenv.axon_hooks")
    mod._hook = _hook
    mod.get_axon_ntff_profile_hook = lambda: _hook
    mod.set_axon_ntff_profile_hook = lambda h: None
    _sys.modules["antenv.axon_hooks"] = mod


def kernel(query, mem_questions, mem_responses, mem_traces, mem_strengths,
           top_k, _trace=False, _results_box=None):
    from concourse import bass_utils

    if _trace:
        _install_ntff_shim()

    k = int(top_k)
    in_maps, ns = make_in_maps(
        query, mem_questions, mem_responses, mem_traces, mem_strengths)
    nc = _get_program(ns)
    res = bass_utils.run_bass_kernel_spmd(
        nc, in_maps, core_ids=list(range(N_CORES)), trace=_trace)
    if _results_box is not None:
        _results_box.append(res)
    return merge_candidates(res.results, ns, k)
